# revision 18
# baseline (speedup 1.0000x reference)
"""PointConvolution (8-neighbor shifted diffs + 1x1 conv) as a single 3x3 conv,
run data-parallel across 8 TRN2 NeuronCores via Bass/Tile.

Math: out[o,h,w] = sum_k sum_c W[o,3k+c] * (xpad[c,h+ik,w+jk] - x[c,h,w]) + b[o]
    = sum_{c,i,j} K3[o,c,i,j] * xpad[c,h+i,w+j] + b[o]
  where K3 gets W at the 8 non-center taps and -sum(W over taps) at center.

Device scheme per core (2 images), v12 (bf16 matmuls, 4x32 PE row tiling,
minimal-HBM input):
  - Output rows in chunks of 32 = 8 groups of G=4 rows. M=128 PSUM
    partitions = (g, o). The j column shifts are NOT materialized: each
    group runs KS=3 bf16 matmuls that accumulate in PSUM, with the moving
    operand's column window shifted by j (the padded row is 514 wide, so
    [j : j+512] always fits). bf16 (not fp16!) because the PE streams bf16
    at 1 col/cycle; fp16 measured 743ns vs bf16's documented ~379ns per
    512-col matmul.
  - Row-window trick kills im2row row duplication: per chunk, quadrant q'
    (SBUF partitions 32q'..32q'+29) holds the 10 distinct input rows for
    groups t = 2q', 2q'+1 as partitions 3*rr + c. Group selection lives in
    the STATIONARY: w[tg][j] is [30, 128] with the 18 live rows placed at
    offset 12*tg, zeros elsewhere - so every matmul AP starts exactly at a
    32-aligned quadrant base, and the moving AP is the full quadrant.
  - PE runs 32x128 row-tiled: 4 tiles (0/32/64/96), one per quadrant,
    streaming concurrently. Matmul issue interleaves q' so all 4 tiles stay
    busy (also keeps the PE continuously fed - HAM throttles the array to
    half clock if it idles).
  - PSUM: one [128, 2, 512] tile (2 banks) per (par, tg, half); quadrants
    2h, 2h+1 write ps[:, q'%2, :]; bufs=4 -> 8 banks. The two halves of a
    step drain CONCURRENTLY on DVE (half 0) and ACT (half 1) - with a
    serial per-step drain the chain mm -> sem -> drain -> sem -> mm(k+2)
    was the pacing loop (v12 measured 2.7us/step vs 1.9us of stream). The
    drain dst is a strided 3-dim AP since t = 2q' + tg interleaves rows.
  - Input: ONE gpsimd DMA per chunk-pair, [128, 2*514] bf16 (263KB, 2056B
    per-partition lines), prefetched 2 pairs deep -> 4.2MB/core.
    Output: ONE 2MB DMA per pair ([128, 2, 4096] fp16 = 16KB contiguous per
    partition), alternating between the two HWDGE queues.
  - HBM traffic/core: 4.2MB in + 33.5MB out = 37.7MB -> ~105us roofline at
    358 GB/s. Host transposes + upcasts fp16->fp32 during unshard.
"""

import ml_dtypes
import numpy as np

import concourse.bacc as bacc
import concourse.bass as bass
import concourse.tile as tile
from concourse import mybir
from concourse.bass_utils import run_bass_kernel_spmd

# Problem constants (hardcoded per harness contract)
B, C, H, W_DIM, OUT = 16, 3, 512, 512, 32
KS, P = 3, 1
NCORES = 8
NB = B // NCORES          # images per core = 2
Hp, Wp = H + 2 * P, W_DIM + 2 * P   # 514, 514

G = 4                     # output rows per matmul group
S = G + KS - 1            # input rows per group window = 6
TB = 8                    # groups per chunk (32 output rows)
CH = G * TB               # 32 output rows per chunk
NCHUNK = H // CH          # 16 chunks per image
NPC = NCHUNK // 2         # chunk pairs per image = 8
NPAIR = NB * NPC          # chunk pairs per core = 16
KQ = 30                   # contraction rows per quadrant (10 rows x 3 chan)
M = G * OUT               # 128 output partitions
OBF = TB * W_DIM          # 4096 free cols per chunk in the output tile

F32 = mybir.dt.float32
F16 = mybir.dt.float16
BF16 = mybir.dt.bfloat16
NP_BF16 = ml_dtypes.bfloat16


def _coords():
    i, j = np.meshgrid(np.arange(KS), np.arange(KS))
    coords = np.dstack((i.reshape(-1), j.reshape(-1)))[0]
    return coords[np.any(coords != P, axis=1)]


def _build_weights(W, b):
    K3 = np.zeros((OUT, C, KS, KS), np.float32)
    Wr = W.reshape(OUT, 8, C)
    for k, (i, j) in enumerate(_coords()):
        K3[:, :, i, j] += Wr[:, k, :]
    K3[:, :, P, P] = -Wr.sum(axis=1)

    # wts[tg, j, 12tg + 3s + c, 32g + o] = K3[o, c, s-g, j] when 0 <= s-g < KS
    wts = np.zeros((2, KS, KQ, M), np.float32)
    for tg in range(2):
        for j in range(KS):
            for s in range(S):
                for c in range(C):
                    for g in range(G):
                        i = s - g
                        if 0 <= i < KS:
                            wts[tg, j, 12 * tg + 3 * s + c,
                                OUT * g: OUT * (g + 1)] = K3[:, c, i, j]
    # -> [KQ, (tg,j) blocks of M cols] for a single DMA per quadrant base
    wt = wts.transpose(2, 0, 1, 3).reshape(KQ, 2 * KS * M)
    bias = np.tile(b.astype(np.float32), G).reshape(M, 1)
    return wt.astype(NP_BF16), bias


def _build_xin(x):
    """[B,C,H,W] fp32 -> [B, NPC, 128, 2*Wp] bf16: per chunk pair, partition
    32q' + 3rr + c holds padded row 32*chunk + 8q' + rr (rr in 0..9) of
    channel c, for both pair chunks side by side in the free dim."""
    x16 = np.ascontiguousarray(x, np.float32).astype(NP_BF16)
    xpad = np.pad(x16, ((0, 0), (0, 0), (P, P), (P, P)))  # [B,C,514,514]
    pc = np.arange(NPC)[:, None, None, None]
    qq = np.arange(4)[None, :, None, None]
    rr = np.arange(10)[None, None, :, None]
    par = np.arange(2)[None, None, None, :]
    rows = CH * (2 * pc + par) + 8 * qq + rr        # [NPC, 4, 10, 2]
    g = xpad[:, :, rows, :]                          # [B, C, NPC, 4, 10, 2, Wp]
    g = g.transpose(0, 2, 3, 4, 1, 5, 6)             # [B, NPC, 4, 10, C, 2, Wp]
    arr = np.zeros((B, NPC, 4, 32, 2, Wp), NP_BF16)
    arr[:, :, :, :KQ] = g.reshape(B, NPC, 4, KQ, 2, Wp)
    return arr.reshape(B, NPC, 128, 2 * Wp)


def _build_bass():
    # Bacc (not plain Bass): its compile() runs move_matmul_waits_to_ldweights
    # and generate_event_semaphores, required because TRN2 instructions take
    # at most one semaphore wait.
    nc = bacc.Bacc("TRN2")
    x_d = nc.declare_dram_parameter("xin", [NB, NPC, 128, 2 * Wp], BF16, isOutput=False)
    wt_d = nc.declare_dram_parameter("wt", [KQ, 2 * KS * M], BF16, isOutput=False)
    b_d = nc.declare_dram_parameter("bias", [M, 1], F32, isOutput=False)
    out_d = nc.declare_dram_parameter(
        "out", [NB, NPC, M, 2, OBF], F16, isOutput=True
    )

    with tile.TileContext(nc) as tc:
        with (
            tc.tile_pool(name="wpool", bufs=1) as wpool,
            tc.tile_pool(name="xpool", bufs=6) as xpool,
            tc.tile_pool(name="opool", bufs=3) as opool,
            tc.tile_pool(name="psum", bufs=2, space=bass.MemorySpace.PSUM) as ppool,
        ):
            # Stationaries replicated at all 4 quadrant bases; col block
            # (3*tg + j) * M selects the group-offset/shift variant.
            wsb = wpool.tile([96 + KQ, 2 * KS * M], BF16)
            for q in range(4):
                nc.scalar.dma_start(wsb[32 * q: 32 * q + KQ, :], wt_d[:])
            bsb = wpool.tile([M, 1], F32)
            nc.scalar.dma_start(bsb[:], b_d[:])

            def load_pair(pair):
                xin = xpool.tile([128, 2 * Wp], BF16)
                src = bass.AP(
                    x_d, pair * 128 * 2 * Wp, [[2 * Wp, 128], [1, 2 * Wp]]
                )
                nc.gpsimd.dma_start(xin[:], src)
                return xin

            def process_pair(pair, xin):
                ob = opool.tile([M, 2, OBF], F16)
                for tg in range(2):
                    for h in range(2):             # tile pair {2h, 2h+1}
                        # one psum tile per pair-chunk parity; each step
                        # covers 12 matmuls on 2 array tiles, so the 2-deep
                        # generation ping-pong has ~2 steps of slack over
                        # the drain+sem chain (the PE's 64-deep window and
                        # the next step's tiles keep the array streaming)
                        pss = [ppool.tile([M, 2, W_DIM], F32, name=f"ps{par}")
                               for par in range(2)]
                        for qq in range(2):
                            q = 2 * h + qq
                            for par in range(2):
                                for j in range(KS):
                                    nc.tensor.matmul(
                                        pss[par][:, qq, :],
                                        wsb[32 * q: 32 * q + KQ,
                                            (KS * tg + j) * M: (KS * tg + j + 1) * M],
                                        xin[32 * q: 32 * q + KQ,
                                            par * Wp + j: par * Wp + j + W_DIM],
                                        start=(j == 0),
                                        stop=(j == KS - 1),
                                        tile_position=(32 * q, 0),
                                    )
                        # two concurrent drains (DVE par=0 / ACT par=1);
                        # dst rows t = 2q'+tg are 512-col blocks, stride 1024
                        for par in range(2):
                            dst = bass.AP(
                                ob.tensor,
                                ob.offset + par * OBF + (4 * h + tg) * W_DIM,
                                [[2 * OBF, M], [2 * W_DIM, 2], [1, W_DIM]],
                            )
                            if par == 0:
                                nc.vector.tensor_scalar_add(
                                    dst, pss[par][:, :, :], bsb[:])
                            else:
                                nc.scalar.add(dst, pss[par][:, :, :], bsb[:])

                odst = bass.AP(
                    out_d, pair * M * 2 * OBF, [[2 * OBF, M], [1, 2 * OBF]]
                )
                qout = nc.scalar if pair % 2 == 0 else nc.sync
                qout.dma_start(odst, ob[:])

            # three-stage input prefetch: loads run three pairs ahead
            DEPTH = 3
            tiles = [load_pair(p) for p in range(DEPTH)]
            for pair in range(DEPTH, NPAIR):
                tiles.append(load_pair(pair))
                process_pair(pair - DEPTH, tiles.pop(0))
            for k in range(DEPTH):
                process_pair(NPAIR - DEPTH + k, tiles.pop(0))
    nc.finalize()
    return nc


_NC_CACHE = None


def _get_nc():
    global _NC_CACHE
    if _NC_CACHE is None:
        _NC_CACHE = _build_bass()
    return _NC_CACHE


def kernel(x, W, b, trace=False, **trace_kw):
    xin = _build_xin(np.asarray(x, np.float32))
    wt, bias = _build_weights(np.asarray(W, np.float32), np.asarray(b, np.float32))
    in_maps = [
        {"xin": xin[NB * m: NB * (m + 1)], "wt": wt, "bias": bias}
        for m in range(NCORES)
    ]
    res = run_bass_kernel_spmd(
        _get_nc(), in_maps, list(range(NCORES)), trace=trace, **trace_kw
    )
    # Device layout [NB, pc, 32g+o, par, (t, w)] -> [B, OUT, H, W]:
    # row = 32*(2*pc + par) + 4*t + g
    parts = []
    for m in range(NCORES):
        o = res.results[m]["out"].reshape(NB, NPC, G, OUT, 2, TB, W_DIM)
        parts.append(
            o.transpose(0, 3, 1, 4, 5, 2, 6).reshape(NB, OUT, H, W_DIM)
        )
    out = np.ascontiguousarray(np.concatenate(parts, axis=0)).astype(np.float32)
    if trace:
        kernel.last_results = res
    return out


# revision 19
# speedup vs baseline: 1.3378x; 1.3378x over previous
"""PointConvolution (8-neighbor shifted diffs + 1x1 conv) as a single 3x3 conv,
run data-parallel across 8 TRN2 NeuronCores via Bass/Tile.

Math: out[o,h,w] = sum_k sum_c W[o,3k+c] * (xpad[c,h+ik,w+jk] - x[c,h,w]) + b[o]
    = sum_{c,i,j} K3[o,c,i,j] * xpad[c,h+i,w+j] + b[o]
  where K3 gets W at the 8 non-center taps and -sum(W over taps) at center.

Device scheme per core (2 images), v12 (bf16 matmuls, 4x32 PE row tiling,
minimal-HBM input):
  - Output rows in chunks of 32 = 8 groups of G=4 rows. M=128 PSUM
    partitions = (g, o). The j column shifts are NOT materialized: each
    group runs KS=3 bf16 matmuls that accumulate in PSUM, with the moving
    operand's column window shifted by j (the padded row is 514 wide, so
    [j : j+512] always fits). bf16 (not fp16!) because the PE streams bf16
    at 1 col/cycle; fp16 measured 743ns vs bf16's documented ~379ns per
    512-col matmul.
  - Row-window trick kills im2row row duplication: per chunk, quadrant q'
    (SBUF partitions 32q'..32q'+29) holds the 10 distinct input rows for
    groups t = 2q', 2q'+1 as partitions 3*rr + c. Group selection lives in
    the STATIONARY: w[tg][j] is [30, 128] with the 18 live rows placed at
    offset 12*tg, zeros elsewhere - so every matmul AP starts exactly at a
    32-aligned quadrant base, and the moving AP is the full quadrant.
  - PE runs 32x128 row-tiled: 4 tiles (0/32/64/96), one per quadrant,
    streaming concurrently. Matmul issue interleaves q' so all 4 tiles stay
    busy (also keeps the PE continuously fed - HAM throttles the array to
    half clock if it idles).
  - PSUM: one [128, 2, 512] tile (2 banks) per (par, tg, half); quadrants
    2h, 2h+1 write ps[:, q'%2, :]; bufs=4 -> 8 banks. The two halves of a
    step drain CONCURRENTLY on DVE (half 0) and ACT (half 1) - with a
    serial per-step drain the chain mm -> sem -> drain -> sem -> mm(k+2)
    was the pacing loop (v12 measured 2.7us/step vs 1.9us of stream). The
    drain dst is a strided 3-dim AP since t = 2q' + tg interleaves rows.
  - Input: ONE gpsimd DMA per chunk-pair, [128, 2*514] bf16 (263KB, 2056B
    per-partition lines), prefetched 2 pairs deep -> 4.2MB/core.
    Output: ONE 2MB DMA per pair ([128, 2, 4096] fp16 = 16KB contiguous per
    partition), alternating between the two HWDGE queues.
  - HBM traffic/core: 4.2MB in + 33.5MB out = 37.7MB -> ~105us roofline at
    358 GB/s. Host transposes + upcasts fp16->fp32 during unshard.
"""

import ml_dtypes
import numpy as np

import concourse.bacc as bacc
import concourse.bass as bass
import concourse.tile as tile
from concourse import mybir
from concourse.bass_utils import run_bass_kernel_spmd

# Problem constants (hardcoded per harness contract)
B, C, H, W_DIM, OUT = 16, 3, 512, 512, 32
KS, P = 3, 1
NCORES = 8
NB = B // NCORES          # images per core = 2
Hp, Wp = H + 2 * P, W_DIM + 2 * P   # 514, 514

G = 4                     # output rows per matmul group
S = G + KS - 1            # input rows per group window = 6
TB = 8                    # groups per chunk (32 output rows)
CH = G * TB               # 32 output rows per chunk
NCHUNK = H // CH          # 16 chunks per image
NPC = NCHUNK // 2         # chunk pairs per image = 8
NPAIR = NB * NPC          # chunk pairs per core = 16
KQ = 30                   # contraction rows per quadrant (10 rows x 3 chan)
M = G * OUT               # 128 output partitions
OBF = TB * W_DIM          # 4096 free cols per chunk in the output tile

F32 = mybir.dt.float32
F16 = mybir.dt.float16
BF16 = mybir.dt.bfloat16
NP_BF16 = ml_dtypes.bfloat16


def _coords():
    i, j = np.meshgrid(np.arange(KS), np.arange(KS))
    coords = np.dstack((i.reshape(-1), j.reshape(-1)))[0]
    return coords[np.any(coords != P, axis=1)]


def _build_weights(W, b):
    K3 = np.zeros((OUT, C, KS, KS), np.float32)
    Wr = W.reshape(OUT, 8, C)
    for k, (i, j) in enumerate(_coords()):
        K3[:, :, i, j] += Wr[:, k, :]
    K3[:, :, P, P] = -Wr.sum(axis=1)

    # wts[tg, j, 12tg + 3s + c, 32g + o] = K3[o, c, s-g, j] when 0 <= s-g < KS
    wts = np.zeros((2, KS, KQ, M), np.float32)
    for tg in range(2):
        for j in range(KS):
            for s in range(S):
                for c in range(C):
                    for g in range(G):
                        i = s - g
                        if 0 <= i < KS:
                            wts[tg, j, 12 * tg + 3 * s + c,
                                OUT * g: OUT * (g + 1)] = K3[:, c, i, j]
    # -> [KQ, (tg,j) blocks of M cols] for a single DMA per quadrant base
    wt = wts.transpose(2, 0, 1, 3).reshape(KQ, 2 * KS * M)
    bias = np.tile(b.astype(np.float32), G).reshape(M, 1)
    return wt.astype(NP_BF16), bias


def _build_xin(x):
    """[B,C,H,W] fp32 -> [B, NPC, 128, 2*Wp] bf16: per chunk pair, partition
    32q' + 3rr + c holds padded row 32*chunk + 8q' + rr (rr in 0..9) of
    channel c, for both pair chunks side by side in the free dim."""
    x16 = np.ascontiguousarray(x, np.float32).astype(NP_BF16)
    xpad = np.pad(x16, ((0, 0), (0, 0), (P, P), (P, P)))  # [B,C,514,514]
    pc = np.arange(NPC)[:, None, None, None]
    qq = np.arange(4)[None, :, None, None]
    rr = np.arange(10)[None, None, :, None]
    par = np.arange(2)[None, None, None, :]
    rows = CH * (2 * pc + par) + 8 * qq + rr        # [NPC, 4, 10, 2]
    g = xpad[:, :, rows, :]                          # [B, C, NPC, 4, 10, 2, Wp]
    g = g.transpose(0, 2, 3, 4, 1, 5, 6)             # [B, NPC, 4, 10, C, 2, Wp]
    arr = np.zeros((B, NPC, 4, 32, 2, Wp), NP_BF16)
    arr[:, :, :, :KQ] = g.reshape(B, NPC, 4, KQ, 2, Wp)
    return arr.reshape(B, NPC, 128, 2 * Wp)


def _build_bass():
    # Bacc (not plain Bass): its compile() runs move_matmul_waits_to_ldweights
    # and generate_event_semaphores, required because TRN2 instructions take
    # at most one semaphore wait.
    nc = bacc.Bacc("TRN2")
    x_d = nc.declare_dram_parameter("xin", [NB, NPC, 128, 2 * Wp], BF16, isOutput=False)
    wt_d = nc.declare_dram_parameter("wt", [KQ, 2 * KS * M], BF16, isOutput=False)
    b_d = nc.declare_dram_parameter("bias", [M, 1], F32, isOutput=False)
    out_d = nc.declare_dram_parameter(
        "out", [NB, NPC, M, 2, OBF], F16, isOutput=True
    )

    with tile.TileContext(nc) as tc:
        with (
            tc.tile_pool(name="wpool", bufs=1) as wpool,
            tc.tile_pool(name="xpool", bufs=6) as xpool,
            tc.tile_pool(name="opool", bufs=3) as opool,
            tc.tile_pool(name="psum", bufs=2, space=bass.MemorySpace.PSUM) as ppool,
        ):
            # Stationaries replicated at all 4 quadrant bases; col block
            # (3*tg + j) * M selects the group-offset/shift variant.
            wsb = wpool.tile([96 + KQ, 2 * KS * M], BF16)
            for q in range(4):
                nc.scalar.dma_start(wsb[32 * q: 32 * q + KQ, :], wt_d[:])
            bsb = wpool.tile([M, 1], F32)
            nc.scalar.dma_start(bsb[:], b_d[:])

            def load_pair(pair):
                xin = xpool.tile([128, 2 * Wp], BF16)
                src = bass.AP(
                    x_d, pair * 128 * 2 * Wp, [[2 * Wp, 128], [1, 2 * Wp]]
                )
                nc.gpsimd.dma_start(xin[:], src)
                return xin

            def process_pair(pair, xin):
                ob = opool.tile([M, 2, OBF], F16)
                for par in range(2):
                    coff = par * Wp
                    for tg in range(2):
                        pss = [ppool.tile([M, 2, W_DIM], F32, name=f"ps{h}")
                               for h in range(2)]
                        for j in range(KS):
                            for q in range(4):     # round-robin the 4 tiles
                                nc.tensor.matmul(
                                    pss[q // 2][:, q % 2, :],
                                    wsb[32 * q: 32 * q + KQ,
                                        (KS * tg + j) * M: (KS * tg + j + 1) * M],
                                    xin[32 * q: 32 * q + KQ,
                                        coff + j: coff + j + W_DIM],
                                    start=(j == 0),
                                    stop=(j == KS - 1),
                                    tile_position=(32 * q, 0),
                                )
                        # two concurrent drains per (par, tg): dst rows
                        # t = 2q'+tg are 512-col blocks at stride 1024
                        for h in range(2):
                            dst = bass.AP(
                                ob.tensor,
                                ob.offset + par * OBF + (4 * h + tg) * W_DIM,
                                [[2 * OBF, M], [2 * W_DIM, 2], [1, W_DIM]],
                            )
                            if h == 0:
                                nc.vector.tensor_scalar_add(
                                    dst, pss[h][:, :, :], bsb[:])
                            else:
                                nc.scalar.add(dst, pss[h][:, :, :], bsb[:])
                    # per-par 1MB output DMA: fires as soon as this chunk's
                    # 4 drains land (earlier queue start, shorter trigger
                    # stalls on the issuing engine than a per-pair 2MB DMA)
                    odst = bass.AP(
                        out_d,
                        (pair * M * 2 + par) * OBF,
                        [[2 * OBF, M], [1, OBF]],
                    )
                    qout = nc.scalar if (2 * pair + par) % 2 == 0 else nc.sync
                    qout.dma_start(odst, ob[:, par, :])

            # three-stage input prefetch: loads run three pairs ahead
            DEPTH = 3
            tiles = [load_pair(p) for p in range(DEPTH)]
            for pair in range(DEPTH, NPAIR):
                tiles.append(load_pair(pair))
                process_pair(pair - DEPTH, tiles.pop(0))
            for k in range(DEPTH):
                process_pair(NPAIR - DEPTH + k, tiles.pop(0))
    nc.finalize()
    return nc


_NC_CACHE = None


def _get_nc():
    global _NC_CACHE
    if _NC_CACHE is None:
        _NC_CACHE = _build_bass()
    return _NC_CACHE


def kernel(x, W, b, trace=False, **trace_kw):
    xin = _build_xin(np.asarray(x, np.float32))
    wt, bias = _build_weights(np.asarray(W, np.float32), np.asarray(b, np.float32))
    in_maps = [
        {"xin": xin[NB * m: NB * (m + 1)], "wt": wt, "bias": bias}
        for m in range(NCORES)
    ]
    res = run_bass_kernel_spmd(
        _get_nc(), in_maps, list(range(NCORES)), trace=trace, **trace_kw
    )
    # Device layout [NB, pc, 32g+o, par, (t, w)] -> [B, OUT, H, W]:
    # row = 32*(2*pc + par) + 4*t + g
    parts = []
    for m in range(NCORES):
        o = res.results[m]["out"].reshape(NB, NPC, G, OUT, 2, TB, W_DIM)
        parts.append(
            o.transpose(0, 3, 1, 4, 5, 2, 6).reshape(NB, OUT, H, W_DIM)
        )
    out = np.ascontiguousarray(np.concatenate(parts, axis=0)).astype(np.float32)
    if trace:
        kernel.last_results = res
    return out


# revision 23
# speedup vs baseline: 1.3526x; 1.0110x over previous
"""PointConvolution (8-neighbor shifted diffs + 1x1 conv) as a single 3x3 conv,
run data-parallel across 8 TRN2 NeuronCores via Bass/Tile.

Math: out[o,h,w] = sum_k sum_c W[o,3k+c] * (xpad[c,h+ik,w+jk] - x[c,h,w]) + b[o]
    = sum_{c,i,j} K3[o,c,i,j] * xpad[c,h+i,w+j] + b[o]
  where K3 gets W at the 8 non-center taps and -sum(W over taps) at center.

Device scheme per core (2 images), v12 (bf16 matmuls, 4x32 PE row tiling,
minimal-HBM input):
  - Output rows in chunks of 32 = 8 groups of G=4 rows. M=128 PSUM
    partitions = (g, o). The j column shifts are NOT materialized: each
    group runs KS=3 bf16 matmuls that accumulate in PSUM, with the moving
    operand's column window shifted by j (the padded row is 514 wide, so
    [j : j+512] always fits). bf16 (not fp16!) because the PE streams bf16
    at 1 col/cycle; fp16 measured 743ns vs bf16's documented ~379ns per
    512-col matmul.
  - Row-window trick kills im2row row duplication: per chunk, quadrant q'
    (SBUF partitions 32q'..32q'+29) holds the 10 distinct input rows for
    groups t = 2q', 2q'+1 as partitions 3*rr + c. Group selection lives in
    the STATIONARY: w[tg][j] is [30, 128] with the 18 live rows placed at
    offset 12*tg, zeros elsewhere - so every matmul AP starts exactly at a
    32-aligned quadrant base, and the moving AP is the full quadrant.
  - PE runs 32x128 row-tiled: 4 tiles (0/32/64/96), one per quadrant,
    streaming concurrently. Matmul issue interleaves q' so all 4 tiles stay
    busy (also keeps the PE continuously fed - HAM throttles the array to
    half clock if it idles).
  - PSUM: one [128, 2, 512] tile (2 banks) per (par, tg, half); quadrants
    2h, 2h+1 write ps[:, q'%2, :]; bufs=4 -> 8 banks. The two halves of a
    step drain CONCURRENTLY on DVE (half 0) and ACT (half 1) - with a
    serial per-step drain the chain mm -> sem -> drain -> sem -> mm(k+2)
    was the pacing loop (v12 measured 2.7us/step vs 1.9us of stream). The
    drain dst is a strided 3-dim AP since t = 2q' + tg interleaves rows.
  - Input: ONE gpsimd DMA per chunk-pair, [128, 2*514] bf16 (263KB, 2056B
    per-partition lines), prefetched 2 pairs deep -> 4.2MB/core.
    Output: ONE 2MB DMA per pair ([128, 2, 4096] fp16 = 16KB contiguous per
    partition), alternating between the two HWDGE queues.
  - HBM traffic/core: 4.2MB in + 33.5MB out = 37.7MB -> ~105us roofline at
    358 GB/s. Host transposes + upcasts fp16->fp32 during unshard.
"""

import ml_dtypes
import numpy as np

import concourse.bacc as bacc
import concourse.bass as bass
import concourse.tile as tile
from concourse import mybir
from concourse.bass_utils import run_bass_kernel_spmd

# Problem constants (hardcoded per harness contract)
B, C, H, W_DIM, OUT = 16, 3, 512, 512, 32
KS, P = 3, 1
NCORES = 8
NB = B // NCORES          # images per core = 2
Hp, Wp = H + 2 * P, W_DIM + 2 * P   # 514, 514

G = 4                     # output rows per matmul group
S = G + KS - 1            # input rows per group window = 6
TB = 8                    # groups per chunk (32 output rows)
CH = G * TB               # 32 output rows per chunk
NCHUNK = H // CH          # 16 chunks per image
NPC = NCHUNK // 2         # chunk pairs per image = 8
NPAIR = NB * NPC          # chunk pairs per core = 16
KQ = 30                   # contraction rows per quadrant (10 rows x 3 chan)
M = G * OUT               # 128 output partitions
OBF = TB * W_DIM          # 4096 free cols per chunk in the output tile

F32 = mybir.dt.float32
F16 = mybir.dt.float16
BF16 = mybir.dt.bfloat16
NP_BF16 = ml_dtypes.bfloat16


def _coords():
    i, j = np.meshgrid(np.arange(KS), np.arange(KS))
    coords = np.dstack((i.reshape(-1), j.reshape(-1)))[0]
    return coords[np.any(coords != P, axis=1)]


def _build_weights(W, b):
    K3 = np.zeros((OUT, C, KS, KS), np.float32)
    Wr = W.reshape(OUT, 8, C)
    for k, (i, j) in enumerate(_coords()):
        K3[:, :, i, j] += Wr[:, k, :]
    K3[:, :, P, P] = -Wr.sum(axis=1)

    # wts[tg, j, 12tg + 3s + c, 32g + o] = K3[o, c, s-g, j] when 0 <= s-g < KS
    wts = np.zeros((2, KS, KQ, M), np.float32)
    for tg in range(2):
        for j in range(KS):
            for s in range(S):
                for c in range(C):
                    for g in range(G):
                        i = s - g
                        if 0 <= i < KS:
                            wts[tg, j, 12 * tg + 3 * s + c,
                                OUT * g: OUT * (g + 1)] = K3[:, c, i, j]
    # -> [KQ, (tg,j) blocks of M cols] for a single DMA per quadrant base
    wt = wts.transpose(2, 0, 1, 3).reshape(KQ, 2 * KS * M)
    bias = np.tile(b.astype(np.float32), G).reshape(M, 1)
    return wt.astype(NP_BF16), bias


def _build_xin(x):
    """[B,C,H,W] fp32 -> [B, NPC, 128, 2*Wp] bf16: per chunk pair, partition
    32q' + 3rr + c holds padded row 32*chunk + 8q' + rr (rr in 0..9) of
    channel c, for both pair chunks side by side in the free dim."""
    x16 = np.ascontiguousarray(x, np.float32).astype(NP_BF16)
    xpad = np.pad(x16, ((0, 0), (0, 0), (P, P), (P, P)))  # [B,C,514,514]
    pc = np.arange(NPC)[:, None, None, None]
    qq = np.arange(4)[None, :, None, None]
    rr = np.arange(10)[None, None, :, None]
    par = np.arange(2)[None, None, None, :]
    rows = CH * (2 * pc + par) + 8 * qq + rr        # [NPC, 4, 10, 2]
    g = xpad[:, :, rows, :]                          # [B, C, NPC, 4, 10, 2, Wp]
    g = g.transpose(0, 2, 3, 4, 1, 5, 6)             # [B, NPC, 4, 10, C, 2, Wp]
    arr = np.zeros((B, NPC, 4, 32, 2, Wp), NP_BF16)
    arr[:, :, :, :KQ] = g.reshape(B, NPC, 4, KQ, 2, Wp)
    return arr.reshape(B, NPC, 128, 2 * Wp)


def _build_bass():
    # Bacc (not plain Bass): its compile() runs move_matmul_waits_to_ldweights
    # and generate_event_semaphores, required because TRN2 instructions take
    # at most one semaphore wait.
    nc = bacc.Bacc("TRN2")
    x_d = nc.declare_dram_parameter("xin", [NB, NPC, 128, 2 * Wp], BF16, isOutput=False)
    wt_d = nc.declare_dram_parameter("wt", [KQ, 2 * KS * M], BF16, isOutput=False)
    b_d = nc.declare_dram_parameter("bias", [M, 1], F32, isOutput=False)
    out_d = nc.declare_dram_parameter(
        "out", [NB, NPC, M, 2, OBF], F16, isOutput=True
    )

    with tile.TileContext(nc) as tc:
        with (
            tc.tile_pool(name="wpool", bufs=1) as wpool,
            tc.tile_pool(name="xpool", bufs=6) as xpool,
            tc.tile_pool(name="opool", bufs=4) as opool,
            tc.tile_pool(name="psum", bufs=2, space=bass.MemorySpace.PSUM) as ppool,
        ):
            # Stationaries replicated at all 4 quadrant bases; col block
            # (3*tg + j) * M selects the group-offset/shift variant.
            wsb = wpool.tile([96 + KQ, 2 * KS * M], BF16)
            for q in range(4):
                eng = nc.scalar if q % 2 == 0 else nc.sync
                eng.dma_start(wsb[32 * q: 32 * q + KQ, :], wt_d[:])
            bsb = wpool.tile([M, 1], F32)
            nc.sync.dma_start(bsb[:], b_d[:])

            def load_pair(pair):
                xin = xpool.tile([128, 2 * Wp], BF16)
                src = bass.AP(
                    x_d, pair * 128 * 2 * Wp, [[2 * Wp, 128], [1, 2 * Wp]]
                )
                nc.gpsimd.dma_start(xin[:], src)
                return xin

            def process_pair(pair, xin):
                ob = opool.tile([M, 2, OBF], F16)
                for par in range(2):
                    coff = par * Wp
                    for tg in range(2):
                        pss = [ppool.tile([M, 2, W_DIM], F32, name=f"ps{h}")
                               for h in range(2)]
                        for j in range(KS):
                            for q in range(4):     # round-robin the 4 tiles
                                nc.tensor.matmul(
                                    pss[q // 2][:, q % 2, :],
                                    wsb[32 * q: 32 * q + KQ,
                                        (KS * tg + j) * M: (KS * tg + j + 1) * M],
                                    xin[32 * q: 32 * q + KQ,
                                        coff + j: coff + j + W_DIM],
                                    start=(j == 0),
                                    stop=(j == KS - 1),
                                    tile_position=(32 * q, 0),
                                )
                        # two concurrent drains per (par, tg): dst rows
                        # t = 2q'+tg are 512-col blocks at stride 1024
                        for h in range(2):
                            dst = bass.AP(
                                ob.tensor,
                                ob.offset + par * OBF + (4 * h + tg) * W_DIM,
                                [[2 * OBF, M], [2 * W_DIM, 2], [1, W_DIM]],
                            )
                            if h == 0:
                                nc.vector.tensor_scalar_add(
                                    dst, pss[h][:, :, :], bsb[:])
                            else:
                                nc.scalar.add(dst, pss[h][:, :, :], bsb[:])
                return ob

            def flush_pair(pair, ob):
                # deferred 2MB output: issued one pair after its drains, so
                # the trigger's wait is already satisfied (no engine stall)
                # and the queue gets full 2MB-DMA efficiency
                odst = bass.AP(
                    out_d, pair * M * 2 * OBF, [[2 * OBF, M], [1, 2 * OBF]]
                )
                qout = nc.scalar if pair % 2 == 0 else nc.sync
                qout.dma_start(odst, ob[:])

            # three-stage input prefetch: loads run three pairs ahead;
            # outputs flush one pair behind their compute
            DEPTH = 3
            tiles = [load_pair(p) for p in range(DEPTH)]
            obs = {}
            for pair in range(DEPTH, NPAIR):
                tiles.append(load_pair(pair))
                obs[pair - DEPTH] = process_pair(pair - DEPTH, tiles.pop(0))
                if pair - DEPTH - 1 in obs:
                    flush_pair(pair - DEPTH - 1, obs.pop(pair - DEPTH - 1))
            for k in range(DEPTH):
                p = NPAIR - DEPTH + k
                obs[p] = process_pair(p, tiles.pop(0))
                flush_pair(p - 1, obs.pop(p - 1))
            flush_pair(NPAIR - 1, obs.pop(NPAIR - 1))
    nc.finalize()
    return nc


_NC_CACHE = None


def _get_nc():
    global _NC_CACHE
    if _NC_CACHE is None:
        _NC_CACHE = _build_bass()
    return _NC_CACHE


def kernel(x, W, b, trace=False, **trace_kw):
    xin = _build_xin(np.asarray(x, np.float32))
    wt, bias = _build_weights(np.asarray(W, np.float32), np.asarray(b, np.float32))
    in_maps = [
        {"xin": xin[NB * m: NB * (m + 1)], "wt": wt, "bias": bias}
        for m in range(NCORES)
    ]
    res = run_bass_kernel_spmd(
        _get_nc(), in_maps, list(range(NCORES)), trace=trace, **trace_kw
    )
    # Device layout [NB, pc, 32g+o, par, (t, w)] -> [B, OUT, H, W]:
    # row = 32*(2*pc + par) + 4*t + g
    parts = []
    for m in range(NCORES):
        o = res.results[m]["out"].reshape(NB, NPC, G, OUT, 2, TB, W_DIM)
        parts.append(
            o.transpose(0, 3, 1, 4, 5, 2, 6).reshape(NB, OUT, H, W_DIM)
        )
    out = np.ascontiguousarray(np.concatenate(parts, axis=0)).astype(np.float32)
    if trace:
        kernel.last_results = res
    return out


# revision 25
# speedup vs baseline: 1.4536x; 1.0747x over previous
"""PointConvolution (8-neighbor shifted diffs + 1x1 conv) as a single 3x3 conv,
run data-parallel across 8 TRN2 NeuronCores via Bass/Tile.

Math: out[o,h,w] = sum_k sum_c W[o,3k+c] * (xpad[c,h+ik,w+jk] - x[c,h,w]) + b[o]
    = sum_{c,i,j} K3[o,c,i,j] * xpad[c,h+i,w+j] + b[o]
  where K3 gets W at the 8 non-center taps and -sum(W over taps) at center.

Device scheme per core (2 images), v12 (bf16 matmuls, 4x32 PE row tiling,
minimal-HBM input):
  - Output rows in chunks of 32 = 8 groups of G=4 rows. M=128 PSUM
    partitions = (g, o). The j column shifts are NOT materialized: each
    group runs KS=3 bf16 matmuls that accumulate in PSUM, with the moving
    operand's column window shifted by j (the padded row is 514 wide, so
    [j : j+512] always fits). bf16 (not fp16!) because the PE streams bf16
    at 1 col/cycle; fp16 measured 743ns vs bf16's documented ~379ns per
    512-col matmul.
  - Row-window trick kills im2row row duplication: per chunk, quadrant q'
    (SBUF partitions 32q'..32q'+29) holds the 10 distinct input rows for
    groups t = 2q', 2q'+1 as partitions 3*rr + c. Group selection lives in
    the STATIONARY: w[tg][j] is [30, 128] with the 18 live rows placed at
    offset 12*tg, zeros elsewhere - so every matmul AP starts exactly at a
    32-aligned quadrant base, and the moving AP is the full quadrant.
  - PE runs 32x128 row-tiled: 4 tiles (0/32/64/96), one per quadrant,
    streaming concurrently. Matmul issue interleaves q' so all 4 tiles stay
    busy (also keeps the PE continuously fed - HAM throttles the array to
    half clock if it idles).
  - PSUM: one [128, 2, 512] tile (2 banks) per (par, tg, half); quadrants
    2h, 2h+1 write ps[:, q'%2, :]; 2 tiles/step x bufs=2 -> 8 banks. The
    two halves of a step drain CONCURRENTLY on DVE (half 0) and ACT (half
    1) - with a serial per-step drain the chain mm -> sem -> drain -> sem
    -> mm(k+2) was the pacing loop (v12 measured 2.7us/step vs 1.9us of
    stream; stalls also flip the HAM throttle to half clock). The drain
    dst is a strided 3-dim AP since t = 2q' + tg interleaves rows.
  - Input: ONE gpsimd DMA per chunk-pair, [128, 2*514] bf16 (263KB, 2056B
    per-partition lines), prefetched 3 pairs deep -> 4.2MB/core.
    Output: ONE 2MB DMA per pair ([128, 2, 4096] fp16 = 16KB contiguous
    per partition), alternating between the two HWDGE queues and DEFERRED
    by one pair so the trigger's semaphore wait is already satisfied when
    the issuing engine reaches it (no engine stall; measured ~330GB/s per
    queue vs ~200 for eagerly-issued 1MB DMAs).
  - HBM traffic/core: 4.2MB in + 33.5MB out = 37.7MB -> ~105us roofline at
    358 GB/s. Host transposes + upcasts fp16->fp32 during unshard.
    Measured 126.1us (vs 270.9us fp32 v6 baseline, 2.15x).
"""

import ml_dtypes
import numpy as np

import concourse.bacc as bacc
import concourse.bass as bass
import concourse.tile as tile
from concourse import mybir
from concourse.bass_utils import run_bass_kernel_spmd

# Problem constants (hardcoded per harness contract)
B, C, H, W_DIM, OUT = 16, 3, 512, 512, 32
KS, P = 3, 1
NCORES = 8
NB = B // NCORES          # images per core = 2
Hp, Wp = H + 2 * P, W_DIM + 2 * P   # 514, 514

G = 4                     # output rows per matmul group
S = G + KS - 1            # input rows per group window = 6
TB = 8                    # groups per chunk (32 output rows)
CH = G * TB               # 32 output rows per chunk
NCHUNK = H // CH          # 16 chunks per image
NPC = NCHUNK // 2         # chunk pairs per image = 8
NPAIR = NB * NPC          # chunk pairs per core = 16
KQ = 30                   # contraction rows per quadrant (10 rows x 3 chan)
M = G * OUT               # 128 output partitions
OBF = TB * W_DIM          # 4096 free cols per chunk in the output tile

F32 = mybir.dt.float32
F16 = mybir.dt.float16
BF16 = mybir.dt.bfloat16
NP_BF16 = ml_dtypes.bfloat16


def _coords():
    i, j = np.meshgrid(np.arange(KS), np.arange(KS))
    coords = np.dstack((i.reshape(-1), j.reshape(-1)))[0]
    return coords[np.any(coords != P, axis=1)]


def _build_weights(W, b):
    K3 = np.zeros((OUT, C, KS, KS), np.float32)
    Wr = W.reshape(OUT, 8, C)
    for k, (i, j) in enumerate(_coords()):
        K3[:, :, i, j] += Wr[:, k, :]
    K3[:, :, P, P] = -Wr.sum(axis=1)

    # wts[tg, j, 12tg + 3s + c, 32g + o] = K3[o, c, s-g, j] when 0 <= s-g < KS
    wts = np.zeros((2, KS, KQ, M), np.float32)
    for tg in range(2):
        for j in range(KS):
            for s in range(S):
                for c in range(C):
                    for g in range(G):
                        i = s - g
                        if 0 <= i < KS:
                            wts[tg, j, 12 * tg + 3 * s + c,
                                OUT * g: OUT * (g + 1)] = K3[:, c, i, j]
    # -> [KQ, (tg,j) blocks of M cols] for a single DMA per quadrant base
    wt = wts.transpose(2, 0, 1, 3).reshape(KQ, 2 * KS * M)
    bias = np.tile(b.astype(np.float32), G).reshape(M, 1)
    return wt.astype(NP_BF16), bias


def _build_xin(x):
    """[B,C,H,W] fp32 -> [B, NPC, 128, 2*Wp] bf16: per chunk pair, partition
    32q' + 3rr + c holds padded row 32*chunk + 8q' + rr (rr in 0..9) of
    channel c, for both pair chunks side by side in the free dim."""
    x16 = np.ascontiguousarray(x, np.float32).astype(NP_BF16)
    xpad = np.pad(x16, ((0, 0), (0, 0), (P, P), (P, P)))  # [B,C,514,514]
    pc = np.arange(NPC)[:, None, None, None]
    qq = np.arange(4)[None, :, None, None]
    rr = np.arange(10)[None, None, :, None]
    par = np.arange(2)[None, None, None, :]
    rows = CH * (2 * pc + par) + 8 * qq + rr        # [NPC, 4, 10, 2]
    g = xpad[:, :, rows, :]                          # [B, C, NPC, 4, 10, 2, Wp]
    g = g.transpose(0, 2, 3, 4, 1, 5, 6)             # [B, NPC, 4, 10, C, 2, Wp]
    arr = np.zeros((B, NPC, 4, 32, 2, Wp), NP_BF16)
    arr[:, :, :, :KQ] = g.reshape(B, NPC, 4, KQ, 2, Wp)
    return arr.reshape(B, NPC, 128, 2 * Wp)


def _build_bass():
    # Bacc (not plain Bass): its compile() runs move_matmul_waits_to_ldweights
    # and generate_event_semaphores, required because TRN2 instructions take
    # at most one semaphore wait.
    nc = bacc.Bacc("TRN2")
    x_d = nc.declare_dram_parameter("xin", [NB, NPC, 128, 2 * Wp], BF16, isOutput=False)
    wt_d = nc.declare_dram_parameter("wt", [KQ, 2 * KS * M], BF16, isOutput=False)
    b_d = nc.declare_dram_parameter("bias", [M, 1], F32, isOutput=False)
    out_d = nc.declare_dram_parameter(
        "out", [NB, NPC, M, 2, OBF], F16, isOutput=True
    )

    with tile.TileContext(nc) as tc:
        with (
            tc.tile_pool(name="wpool", bufs=1) as wpool,
            tc.tile_pool(name="xpool", bufs=6) as xpool,
            tc.tile_pool(name="opool", bufs=4) as opool,
            tc.tile_pool(name="psum", bufs=2, space=bass.MemorySpace.PSUM) as ppool,
        ):
            # Stationaries replicated at all 4 quadrant bases; col block
            # (3*tg + j) * M selects the group-offset/shift variant.
            wsb = wpool.tile([96 + KQ, 2 * KS * M], BF16)
            for q in range(4):
                eng = nc.scalar if q % 2 == 0 else nc.sync
                eng.dma_start(wsb[32 * q: 32 * q + KQ, :], wt_d[:])
            bsb = wpool.tile([M, 1], F32)
            nc.sync.dma_start(bsb[:], b_d[:])

            def load_pair(pair):
                xin = xpool.tile([128, 2 * Wp], BF16)
                src = bass.AP(
                    x_d, pair * 128 * 2 * Wp, [[2 * Wp, 128], [1, 2 * Wp]]
                )
                nc.gpsimd.dma_start(xin[:], src)
                return xin

            def process_pair(pair, xin):
                ob = opool.tile([M, 2, OBF], F16)
                for par in range(2):
                    coff = par * Wp
                    for tg in range(2):
                        pss = [ppool.tile([M, 2, W_DIM], F32, name=f"ps{h}")
                               for h in range(2)]
                        for j in range(KS):
                            for q in range(4):     # round-robin the 4 tiles
                                nc.tensor.matmul(
                                    pss[q // 2][:, q % 2, :],
                                    wsb[32 * q: 32 * q + KQ,
                                        (KS * tg + j) * M: (KS * tg + j + 1) * M],
                                    xin[32 * q: 32 * q + KQ,
                                        coff + j: coff + j + W_DIM],
                                    start=(j == 0),
                                    stop=(j == KS - 1),
                                    tile_position=(32 * q, 0),
                                )
                        # two concurrent drains per (par, tg): dst rows
                        # t = 2q'+tg are 512-col blocks at stride 1024
                        for h in range(2):
                            dst = bass.AP(
                                ob.tensor,
                                ob.offset + par * OBF + (4 * h + tg) * W_DIM,
                                [[2 * OBF, M], [2 * W_DIM, 2], [1, W_DIM]],
                            )
                            if h == 0:
                                nc.vector.tensor_scalar_add(
                                    dst, pss[h][:, :, :], bsb[:])
                            else:
                                nc.scalar.add(dst, pss[h][:, :, :], bsb[:])
                return ob

            def flush_pair(pair, ob):
                # deferred output: issued one pair after its drains, so the
                # trigger's wait is already satisfied (no engine stall).
                # Split per-par across BOTH HWDGE queues: smoother HBM
                # pressure than alternating 2MB bursts, and the final pair's
                # tail flush halves
                for par in range(2):
                    odst = bass.AP(
                        out_d,
                        (pair * M * 2 + par) * OBF,
                        [[2 * OBF, M], [1, OBF]],
                    )
                    qout = nc.scalar if par == 0 else nc.sync
                    qout.dma_start(odst, ob[:, par, :])

            # three-stage input prefetch: loads run three pairs ahead;
            # outputs flush one pair behind their compute
            DEPTH = 3
            tiles = [load_pair(p) for p in range(DEPTH)]
            obs = {}
            for pair in range(DEPTH, NPAIR):
                tiles.append(load_pair(pair))
                obs[pair - DEPTH] = process_pair(pair - DEPTH, tiles.pop(0))
                if pair - DEPTH - 1 in obs:
                    flush_pair(pair - DEPTH - 1, obs.pop(pair - DEPTH - 1))
            for k in range(DEPTH):
                p = NPAIR - DEPTH + k
                obs[p] = process_pair(p, tiles.pop(0))
                flush_pair(p - 1, obs.pop(p - 1))
            flush_pair(NPAIR - 1, obs.pop(NPAIR - 1))
    nc.finalize()
    return nc


_NC_CACHE = None


def _get_nc():
    global _NC_CACHE
    if _NC_CACHE is None:
        _NC_CACHE = _build_bass()
    return _NC_CACHE


def kernel(x, W, b, trace=False, **trace_kw):
    xin = _build_xin(np.asarray(x, np.float32))
    wt, bias = _build_weights(np.asarray(W, np.float32), np.asarray(b, np.float32))
    in_maps = [
        {"xin": xin[NB * m: NB * (m + 1)], "wt": wt, "bias": bias}
        for m in range(NCORES)
    ]
    res = run_bass_kernel_spmd(
        _get_nc(), in_maps, list(range(NCORES)), trace=trace, **trace_kw
    )
    # Device layout [NB, pc, 32g+o, par, (t, w)] -> [B, OUT, H, W]:
    # row = 32*(2*pc + par) + 4*t + g
    parts = []
    for m in range(NCORES):
        o = res.results[m]["out"].reshape(NB, NPC, G, OUT, 2, TB, W_DIM)
        parts.append(
            o.transpose(0, 3, 1, 4, 5, 2, 6).reshape(NB, OUT, H, W_DIM)
        )
    out = np.ascontiguousarray(np.concatenate(parts, axis=0)).astype(np.float32)
    if trace:
        kernel.last_results = res
    return out


# revision 28
# speedup vs baseline: 1.4653x; 1.0080x over previous
"""PointConvolution (8-neighbor shifted diffs + 1x1 conv) as a single 3x3 conv,
run data-parallel across 8 TRN2 NeuronCores via Bass/Tile.

Math: out[o,h,w] = sum_k sum_c W[o,3k+c] * (xpad[c,h+ik,w+jk] - x[c,h,w]) + b[o]
    = sum_{c,i,j} K3[o,c,i,j] * xpad[c,h+i,w+j] + b[o]
  where K3 gets W at the 8 non-center taps and -sum(W over taps) at center.

Device scheme per core (2 images), v12 (bf16 matmuls, 4x32 PE row tiling,
minimal-HBM input):
  - Output rows in chunks of 32 = 8 groups of G=4 rows. M=128 PSUM
    partitions = (g, o). The j column shifts are NOT materialized: each
    group runs KS=3 bf16 matmuls that accumulate in PSUM, with the moving
    operand's column window shifted by j (the padded row is 514 wide, so
    [j : j+512] always fits). bf16 (not fp16!) because the PE streams bf16
    at 1 col/cycle; fp16 measured 743ns vs bf16's documented ~379ns per
    512-col matmul.
  - Row-window trick kills im2row row duplication: per chunk, quadrant q'
    (SBUF partitions 32q'..32q'+29) holds the 10 distinct input rows for
    groups t = 2q', 2q'+1 as partitions 3*rr + c. Group selection lives in
    the STATIONARY: w[tg][j] is [30, 128] with the 18 live rows placed at
    offset 12*tg, zeros elsewhere - so every matmul AP starts exactly at a
    32-aligned quadrant base, and the moving AP is the full quadrant.
  - PE runs 32x128 row-tiled: 4 tiles (0/32/64/96), one per quadrant,
    streaming concurrently. Matmul issue interleaves q' so all 4 tiles stay
    busy (also keeps the PE continuously fed - HAM throttles the array to
    half clock if it idles).
  - PSUM: one [128, 2, 512] tile (2 banks) per (par, tg, half); quadrants
    2h, 2h+1 write ps[:, q'%2, :]; 2 tiles/step x bufs=2 -> 8 banks. The
    two halves of a step drain CONCURRENTLY on DVE (half 0) and ACT (half
    1) - with a serial per-step drain the chain mm -> sem -> drain -> sem
    -> mm(k+2) was the pacing loop (v12 measured 2.7us/step vs 1.9us of
    stream; stalls also flip the HAM throttle to half clock). The drain
    dst is a strided 3-dim AP since t = 2q' + tg interleaves rows.
  - Input: ONE gpsimd DMA per chunk-pair, [128, 2*514] bf16 (263KB, 2056B
    per-partition lines), prefetched 3 pairs deep -> 4.2MB/core.
    Output: ONE 2MB DMA per pair ([128, 2, 4096] fp16 = 16KB contiguous
    per partition), alternating between the two HWDGE queues and DEFERRED
    by one pair so the trigger's semaphore wait is already satisfied when
    the issuing engine reaches it (no engine stall; measured ~330GB/s per
    queue vs ~200 for eagerly-issued 1MB DMAs).
  - HBM traffic/core: 4.2MB in + 33.5MB out = 37.7MB -> ~105us roofline at
    358 GB/s. Host transposes + upcasts fp16->fp32 during unshard.
    Measured 126.1us (vs 270.9us fp32 v6 baseline, 2.15x).
"""

import ml_dtypes
import numpy as np

import concourse.bacc as bacc
import concourse.bass as bass
import concourse.tile as tile
from concourse import mybir
from concourse.bass_utils import run_bass_kernel_spmd

# Problem constants (hardcoded per harness contract)
B, C, H, W_DIM, OUT = 16, 3, 512, 512, 32
KS, P = 3, 1
NCORES = 8
NB = B // NCORES          # images per core = 2
Hp, Wp = H + 2 * P, W_DIM + 2 * P   # 514, 514

G = 4                     # output rows per matmul group
S = G + KS - 1            # input rows per group window = 6
TB = 8                    # groups per chunk (32 output rows)
CH = G * TB               # 32 output rows per chunk
NCHUNK = H // CH          # 16 chunks per image
NPC = NCHUNK // 2         # chunk pairs per image = 8
NPAIR = NB * NPC          # chunk pairs per core = 16
KQ = 30                   # contraction rows per quadrant (10 rows x 3 chan)
M = G * OUT               # 128 output partitions
OBF = TB * W_DIM          # 4096 free cols per chunk in the output tile

F32 = mybir.dt.float32
F16 = mybir.dt.float16
BF16 = mybir.dt.bfloat16
NP_BF16 = ml_dtypes.bfloat16


def _coords():
    i, j = np.meshgrid(np.arange(KS), np.arange(KS))
    coords = np.dstack((i.reshape(-1), j.reshape(-1)))[0]
    return coords[np.any(coords != P, axis=1)]


def _build_weights(W, b):
    K3 = np.zeros((OUT, C, KS, KS), np.float32)
    Wr = W.reshape(OUT, 8, C)
    for k, (i, j) in enumerate(_coords()):
        K3[:, :, i, j] += Wr[:, k, :]
    K3[:, :, P, P] = -Wr.sum(axis=1)

    # wts[tg, j, 12tg + 3s + c, 32g + o] = K3[o, c, s-g, j] when 0 <= s-g < KS
    wts = np.zeros((2, KS, KQ, M), np.float32)
    for tg in range(2):
        for j in range(KS):
            for s in range(S):
                for c in range(C):
                    for g in range(G):
                        i = s - g
                        if 0 <= i < KS:
                            wts[tg, j, 12 * tg + 3 * s + c,
                                OUT * g: OUT * (g + 1)] = K3[:, c, i, j]
    # -> [KQ, (tg,j) blocks of M cols] for a single DMA per quadrant base
    wt = wts.transpose(2, 0, 1, 3).reshape(KQ, 2 * KS * M)
    bias = np.tile(b.astype(np.float32), G).reshape(M, 1)
    return wt.astype(NP_BF16), bias


def _build_xin(x):
    """[B,C,H,W] fp32 -> [B, NPC, 128, 2*Wp] bf16: per chunk pair, partition
    32q' + 3rr + c holds padded row 32*chunk + 8q' + rr (rr in 0..9) of
    channel c, for both pair chunks side by side in the free dim."""
    x16 = np.ascontiguousarray(x, np.float32).astype(NP_BF16)
    xpad = np.pad(x16, ((0, 0), (0, 0), (P, P), (P, P)))  # [B,C,514,514]
    pc = np.arange(NPC)[:, None, None, None]
    qq = np.arange(4)[None, :, None, None]
    rr = np.arange(10)[None, None, :, None]
    par = np.arange(2)[None, None, None, :]
    rows = CH * (2 * pc + par) + 8 * qq + rr        # [NPC, 4, 10, 2]
    g = xpad[:, :, rows, :]                          # [B, C, NPC, 4, 10, 2, Wp]
    g = g.transpose(0, 2, 3, 4, 1, 5, 6)             # [B, NPC, 4, 10, C, 2, Wp]
    arr = np.zeros((B, NPC, 4, 32, 2, Wp), NP_BF16)
    arr[:, :, :, :KQ] = g.reshape(B, NPC, 4, KQ, 2, Wp)
    return arr.reshape(B, NPC, 128, 2 * Wp)


def _build_bass():
    # Bacc (not plain Bass): its compile() runs move_matmul_waits_to_ldweights
    # and generate_event_semaphores, required because TRN2 instructions take
    # at most one semaphore wait.
    nc = bacc.Bacc("TRN2")
    x_d = nc.declare_dram_parameter("xin", [NB, NPC, 128, 2 * Wp], BF16, isOutput=False)
    wt_d = nc.declare_dram_parameter("wt", [KQ, 2 * KS * M], BF16, isOutput=False)
    b_d = nc.declare_dram_parameter("bias", [M, 1], F32, isOutput=False)
    out_d = nc.declare_dram_parameter(
        "out", [NB, NPC, M, 2, OBF], F16, isOutput=True
    )

    with tile.TileContext(nc) as tc:
        with (
            tc.tile_pool(name="wpool", bufs=1) as wpool,
            tc.tile_pool(name="xpool", bufs=6) as xpool,
            tc.tile_pool(name="opool", bufs=4) as opool,
            tc.tile_pool(name="psum", bufs=2, space=bass.MemorySpace.PSUM) as ppool,
        ):
            # Stationaries replicated at all 4 quadrant bases; col block
            # (3*tg + j) * M selects the group-offset/shift variant.
            wsb = wpool.tile([96 + KQ, 2 * KS * M], BF16)
            for q in range(4):
                eng = nc.scalar if q % 2 == 0 else nc.sync
                eng.dma_start(wsb[32 * q: 32 * q + KQ, :], wt_d[:])
            bsb = wpool.tile([M, 1], F32)
            nc.sync.dma_start(bsb[:], b_d[:])

            def load_pair(pair):
                xin = xpool.tile([128, 2 * Wp], BF16)
                src = bass.AP(
                    x_d, pair * 128 * 2 * Wp, [[2 * Wp, 128], [1, 2 * Wp]]
                )
                nc.gpsimd.dma_start(xin[:], src)
                return xin

            def process_pair(pair, xin):
                # separate per-par output tiles: each par's flush then
                # depends only on its own 4 drains (tile-granular deps)
                obs = [opool.tile([M, OBF], F16, name=f"ob{par}")
                       for par in range(2)]
                for par in range(2):
                    ob = obs[par]
                    coff = par * Wp
                    for tg in range(2):
                        pss = [ppool.tile([M, 2, W_DIM], F32, name=f"ps{h}")
                               for h in range(2)]
                        for j in range(KS):
                            for q in range(4):     # round-robin the 4 tiles
                                nc.tensor.matmul(
                                    pss[q // 2][:, q % 2, :],
                                    wsb[32 * q: 32 * q + KQ,
                                        (KS * tg + j) * M: (KS * tg + j + 1) * M],
                                    xin[32 * q: 32 * q + KQ,
                                        coff + j: coff + j + W_DIM],
                                    start=(j == 0),
                                    stop=(j == KS - 1),
                                    tile_position=(32 * q, 0),
                                )
                        # two concurrent drains per (par, tg): dst rows
                        # t = 2q'+tg are 512-col blocks at stride 1024.
                        # Engine for each half alternates by step parity to
                        # balance DVE/ACT load (ACT also runs DMA triggers)
                        for h in range(2):
                            dst = bass.AP(
                                ob.tensor,
                                ob.offset + (4 * h + tg) * W_DIM,
                                [[OBF, M], [2 * W_DIM, 2], [1, W_DIM]],
                            )
                            if h == (par + tg) % 2:
                                nc.vector.tensor_scalar_add(
                                    dst, pss[h][:, :, :], bsb[:])
                            else:
                                nc.scalar.add(dst, pss[h][:, :, :], bsb[:])
                return obs

            def flush_pair(pair, obs):
                # deferred output: issued one pair after its drains, so the
                # trigger's wait is already satisfied (no engine stall).
                # Split per-par across BOTH HWDGE queues: smoother HBM
                # pressure than alternating 2MB bursts, and the final pair's
                # tail flush halves
                for par in range(2):
                    odst = bass.AP(
                        out_d,
                        (pair * M * 2 + par) * OBF,
                        [[2 * OBF, M], [1, OBF]],
                    )
                    qout = nc.scalar if par == 0 else nc.sync
                    qout.dma_start(odst, obs[par][:])

            # three-stage input prefetch: loads run three pairs ahead;
            # outputs flush one pair behind their compute
            DEPTH = 3
            tiles = [load_pair(p) for p in range(DEPTH)]
            obs = {}
            for pair in range(DEPTH, NPAIR):
                tiles.append(load_pair(pair))
                obs[pair - DEPTH] = process_pair(pair - DEPTH, tiles.pop(0))
                if pair - DEPTH - 1 in obs:
                    flush_pair(pair - DEPTH - 1, obs.pop(pair - DEPTH - 1))
            for k in range(DEPTH):
                p = NPAIR - DEPTH + k
                obs[p] = process_pair(p, tiles.pop(0))
                flush_pair(p - 1, obs.pop(p - 1))
            flush_pair(NPAIR - 1, obs.pop(NPAIR - 1))
    nc.finalize()
    return nc


_NC_CACHE = None


def _get_nc():
    global _NC_CACHE
    if _NC_CACHE is None:
        _NC_CACHE = _build_bass()
    return _NC_CACHE


def kernel(x, W, b, trace=False, **trace_kw):
    xin = _build_xin(np.asarray(x, np.float32))
    wt, bias = _build_weights(np.asarray(W, np.float32), np.asarray(b, np.float32))
    in_maps = [
        {"xin": xin[NB * m: NB * (m + 1)], "wt": wt, "bias": bias}
        for m in range(NCORES)
    ]
    res = run_bass_kernel_spmd(
        _get_nc(), in_maps, list(range(NCORES)), trace=trace, **trace_kw
    )
    # Device layout [NB, pc, 32g+o, par, (t, w)] -> [B, OUT, H, W]:
    # row = 32*(2*pc + par) + 4*t + g
    parts = []
    for m in range(NCORES):
        o = res.results[m]["out"].reshape(NB, NPC, G, OUT, 2, TB, W_DIM)
        parts.append(
            o.transpose(0, 3, 1, 4, 5, 2, 6).reshape(NB, OUT, H, W_DIM)
        )
    out = np.ascontiguousarray(np.concatenate(parts, axis=0)).astype(np.float32)
    if trace:
        kernel.last_results = res
    return out


# revision 31
# speedup vs baseline: 1.4726x; 1.0050x over previous
"""PointConvolution (8-neighbor shifted diffs + 1x1 conv) as a single 3x3 conv,
run data-parallel across 8 TRN2 NeuronCores via Bass/Tile.

Math: out[o,h,w] = sum_k sum_c W[o,3k+c] * (xpad[c,h+ik,w+jk] - x[c,h,w]) + b[o]
    = sum_{c,i,j} K3[o,c,i,j] * xpad[c,h+i,w+j] + b[o]
  where K3 gets W at the 8 non-center taps and -sum(W over taps) at center.

Device scheme per core (2 images), v12 (bf16 matmuls, 4x32 PE row tiling,
minimal-HBM input):
  - Output rows in chunks of 32 = 8 groups of G=4 rows. M=128 PSUM
    partitions = (g, o). The j column shifts are NOT materialized: each
    group runs KS=3 bf16 matmuls that accumulate in PSUM, with the moving
    operand's column window shifted by j (the padded row is 514 wide, so
    [j : j+512] always fits). bf16 (not fp16!) because the PE streams bf16
    at 1 col/cycle; fp16 measured 743ns vs bf16's documented ~379ns per
    512-col matmul.
  - Row-window trick kills im2row row duplication: per chunk, quadrant q'
    (SBUF partitions 32q'..32q'+29) holds the 10 distinct input rows for
    groups t = 2q', 2q'+1 as partitions 3*rr + c. Group selection lives in
    the STATIONARY: w[tg][j] is [30, 128] with the 18 live rows placed at
    offset 12*tg, zeros elsewhere - so every matmul AP starts exactly at a
    32-aligned quadrant base, and the moving AP is the full quadrant.
  - PE runs 32x128 row-tiled: 4 tiles (0/32/64/96), one per quadrant,
    streaming concurrently. Matmul issue interleaves q' so all 4 tiles stay
    busy (also keeps the PE continuously fed - HAM throttles the array to
    half clock if it idles).
  - PSUM: one [128, 2, 512] tile (2 banks) per (par, tg, half); quadrants
    2h, 2h+1 write ps[:, q'%2, :]; 2 tiles/step x bufs=2 -> 8 banks. The
    two halves of a step drain CONCURRENTLY on DVE (half 0) and ACT (half
    1) - with a serial per-step drain the chain mm -> sem -> drain -> sem
    -> mm(k+2) was the pacing loop (v12 measured 2.7us/step vs 1.9us of
    stream; stalls also flip the HAM throttle to half clock). The drain
    dst is a strided 3-dim AP since t = 2q' + tg interleaves rows.
  - Input: ONE gpsimd DMA per chunk-pair, [128, 2*514] bf16 (263KB, 2056B
    per-partition lines), prefetched 3 pairs deep -> 4.2MB/core.
    Output: ONE 2MB DMA per pair ([128, 2, 4096] fp16 = 16KB contiguous
    per partition), alternating between the two HWDGE queues and DEFERRED
    by one pair so the trigger's semaphore wait is already satisfied when
    the issuing engine reaches it (no engine stall; measured ~330GB/s per
    queue vs ~200 for eagerly-issued 1MB DMAs).
  - HBM traffic/core: 4.2MB in + 33.5MB out = 37.7MB -> ~105us roofline at
    358 GB/s. Host transposes + upcasts fp16->fp32 during unshard.
    Measured 126.1us (vs 270.9us fp32 v6 baseline, 2.15x).
"""

import ml_dtypes
import numpy as np

import concourse.bacc as bacc
import concourse.bass as bass
import concourse.tile as tile
from concourse import mybir
from concourse.bass_utils import run_bass_kernel_spmd

# Problem constants (hardcoded per harness contract)
B, C, H, W_DIM, OUT = 16, 3, 512, 512, 32
KS, P = 3, 1
NCORES = 8
NB = B // NCORES          # images per core = 2
Hp, Wp = H + 2 * P, W_DIM + 2 * P   # 514, 514

G = 4                     # output rows per matmul group
S = G + KS - 1            # input rows per group window = 6
TB = 8                    # groups per chunk (32 output rows)
CH = G * TB               # 32 output rows per chunk
NCHUNK = H // CH          # 16 chunks per image
NPC = NCHUNK // 2         # chunk pairs per image = 8
NPAIR = NB * NPC          # chunk pairs per core = 16
KQ = 30                   # contraction rows per quadrant (10 rows x 3 chan)
M = G * OUT               # 128 output partitions
OBF = TB * W_DIM          # 4096 free cols per chunk in the output tile

F32 = mybir.dt.float32
F16 = mybir.dt.float16
BF16 = mybir.dt.bfloat16
NP_BF16 = ml_dtypes.bfloat16


def _coords():
    i, j = np.meshgrid(np.arange(KS), np.arange(KS))
    coords = np.dstack((i.reshape(-1), j.reshape(-1)))[0]
    return coords[np.any(coords != P, axis=1)]


def _build_weights(W, b):
    K3 = np.zeros((OUT, C, KS, KS), np.float32)
    Wr = W.reshape(OUT, 8, C)
    for k, (i, j) in enumerate(_coords()):
        K3[:, :, i, j] += Wr[:, k, :]
    K3[:, :, P, P] = -Wr.sum(axis=1)

    # wts[tg, j, 12tg + 3s + c, 32g + o] = K3[o, c, s-g, j] when 0 <= s-g < KS
    wts = np.zeros((2, KS, KQ, M), np.float32)
    for tg in range(2):
        for j in range(KS):
            for s in range(S):
                for c in range(C):
                    for g in range(G):
                        i = s - g
                        if 0 <= i < KS:
                            wts[tg, j, 12 * tg + 3 * s + c,
                                OUT * g: OUT * (g + 1)] = K3[:, c, i, j]
    # -> [KQ, (tg,j) blocks of M cols] for a single DMA per quadrant base
    wt = wts.transpose(2, 0, 1, 3).reshape(KQ, 2 * KS * M)
    bias = np.tile(b.astype(np.float32), G).reshape(M, 1)
    return wt.astype(NP_BF16), bias


def _build_xin(x):
    """[B,C,H,W] fp32 -> [B, NPC, 128, 2*Wp] bf16: per chunk pair, partition
    32q' + 3rr + c holds padded row 32*chunk + 8q' + rr (rr in 0..9) of
    channel c, for both pair chunks side by side in the free dim."""
    x16 = np.ascontiguousarray(x, np.float32).astype(NP_BF16)
    xpad = np.pad(x16, ((0, 0), (0, 0), (P, P), (P, P)))  # [B,C,514,514]
    pc = np.arange(NPC)[:, None, None, None]
    qq = np.arange(4)[None, :, None, None]
    rr = np.arange(10)[None, None, :, None]
    par = np.arange(2)[None, None, None, :]
    rows = CH * (2 * pc + par) + 8 * qq + rr        # [NPC, 4, 10, 2]
    g = xpad[:, :, rows, :]                          # [B, C, NPC, 4, 10, 2, Wp]
    g = g.transpose(0, 2, 3, 4, 1, 5, 6)             # [B, NPC, 4, 10, C, 2, Wp]
    arr = np.zeros((B, NPC, 4, 32, 2, Wp), NP_BF16)
    arr[:, :, :, :KQ] = g.reshape(B, NPC, 4, KQ, 2, Wp)
    return arr.reshape(B, NPC, 128, 2 * Wp)


def _build_bass():
    # Bacc (not plain Bass): its compile() runs move_matmul_waits_to_ldweights
    # and generate_event_semaphores, required because TRN2 instructions take
    # at most one semaphore wait.
    nc = bacc.Bacc("TRN2")
    x_d = nc.declare_dram_parameter("xin", [NB, NPC, 128, 2 * Wp], BF16, isOutput=False)
    wt_d = nc.declare_dram_parameter("wt", [KQ, 2 * KS * M], BF16, isOutput=False)
    b_d = nc.declare_dram_parameter("bias", [M, 1], F32, isOutput=False)
    out_d = nc.declare_dram_parameter(
        "out", [NB, NPC, M, 2, OBF], F16, isOutput=True
    )

    with tile.TileContext(nc) as tc:
        with (
            tc.tile_pool(name="wpool", bufs=1) as wpool,
            tc.tile_pool(name="xpool", bufs=6) as xpool,
            tc.tile_pool(name="opool", bufs=4) as opool,
            tc.tile_pool(name="psum", bufs=2, space=bass.MemorySpace.PSUM) as ppool,
        ):
            # Stationaries replicated at all 4 quadrant bases; col block
            # (3*tg + j) * M selects the group-offset/shift variant.
            wsb = wpool.tile([96 + KQ, 2 * KS * M], BF16)
            for q in range(4):
                eng = nc.scalar if q % 2 == 0 else nc.sync
                eng.dma_start(wsb[32 * q: 32 * q + KQ, :], wt_d[:])
            bsb = wpool.tile([M, 1], F32)
            nc.sync.dma_start(bsb[:], b_d[:])

            def load_pair(pair):
                # separate per-par tiles: the first matmuls of a chunk wait
                # only on their own chunk's input DMA (tile-granular deps)
                xins = []
                for par in range(2):
                    xp = xpool.tile([128, Wp], BF16, name=f"xin{par}")
                    src = bass.AP(
                        x_d,
                        (pair * 128 * 2 + par) * Wp,
                        [[2 * Wp, 128], [1, Wp]],
                    )
                    nc.gpsimd.dma_start(xp[:], src)
                    xins.append(xp)
                return xins

            def process_pair(pair, xins):
                # separate per-par output tiles: each par's flush then
                # depends only on its own 4 drains (tile-granular deps)
                obs = [opool.tile([M, OBF], F16, name=f"ob{par}")
                       for par in range(2)]
                for par in range(2):
                    ob = obs[par]
                    xin = xins[par]
                    coff = 0
                    for tg in range(2):
                        pss = [ppool.tile([M, 2, W_DIM], F32, name=f"ps{h}")
                               for h in range(2)]
                        for j in range(KS):
                            for q in range(4):     # round-robin the 4 tiles
                                nc.tensor.matmul(
                                    pss[q // 2][:, q % 2, :],
                                    wsb[32 * q: 32 * q + KQ,
                                        (KS * tg + j) * M: (KS * tg + j + 1) * M],
                                    xin[32 * q: 32 * q + KQ,
                                        coff + j: coff + j + W_DIM],
                                    start=(j == 0),
                                    stop=(j == KS - 1),
                                    tile_position=(32 * q, 0),
                                )
                        # two concurrent drains per (par, tg): dst rows
                        # t = 2q'+tg are 512-col blocks at stride 1024.
                        # Engine for each half alternates by step parity to
                        # balance DVE/ACT load (ACT also runs DMA triggers)
                        for h in range(2):
                            dst = bass.AP(
                                ob.tensor,
                                ob.offset + (4 * h + tg) * W_DIM,
                                [[OBF, M], [2 * W_DIM, 2], [1, W_DIM]],
                            )
                            if h == (par + tg) % 2:
                                nc.vector.tensor_scalar_add(
                                    dst, pss[h][:, :, :], bsb[:])
                            else:
                                nc.scalar.add(dst, pss[h][:, :, :], bsb[:])
                return obs

            def flush_pair(pair, obs):
                # deferred output: issued one pair after its drains, so the
                # trigger's wait is already satisfied (no engine stall).
                # Split per-par across BOTH HWDGE queues: smoother HBM
                # pressure than alternating 2MB bursts, and the final pair's
                # tail flush halves
                for par in range(2):
                    odst = bass.AP(
                        out_d,
                        (pair * M * 2 + par) * OBF,
                        [[2 * OBF, M], [1, OBF]],
                    )
                    qout = nc.scalar if par == 0 else nc.sync
                    qout.dma_start(odst, obs[par][:])

            # three-stage input prefetch: loads run three pairs ahead;
            # outputs flush one pair behind their compute
            DEPTH = 3
            tiles = [load_pair(p) for p in range(DEPTH)]
            obs = {}
            for pair in range(DEPTH, NPAIR):
                tiles.append(load_pair(pair))
                p = pair - DEPTH
                obs[p] = process_pair(p, tiles.pop(0))
                if p == 0:
                    # eager first flush: starts the output stream ~6us
                    # sooner (engines are idle, so the trigger stall that
                    # deferral avoids mid-pipeline is harmless here)
                    flush_pair(0, obs.pop(0))
                elif p - 1 in obs:
                    flush_pair(p - 1, obs.pop(p - 1))
            for k in range(DEPTH):
                p = NPAIR - DEPTH + k
                obs[p] = process_pair(p, tiles.pop(0))
                flush_pair(p - 1, obs.pop(p - 1))
            flush_pair(NPAIR - 1, obs.pop(NPAIR - 1))
    nc.finalize()
    return nc


_NC_CACHE = None


def _get_nc():
    global _NC_CACHE
    if _NC_CACHE is None:
        _NC_CACHE = _build_bass()
    return _NC_CACHE


def kernel(x, W, b, trace=False, **trace_kw):
    xin = _build_xin(np.asarray(x, np.float32))
    wt, bias = _build_weights(np.asarray(W, np.float32), np.asarray(b, np.float32))
    in_maps = [
        {"xin": xin[NB * m: NB * (m + 1)], "wt": wt, "bias": bias}
        for m in range(NCORES)
    ]
    res = run_bass_kernel_spmd(
        _get_nc(), in_maps, list(range(NCORES)), trace=trace, **trace_kw
    )
    # Device layout [NB, pc, 32g+o, par, (t, w)] -> [B, OUT, H, W]:
    # row = 32*(2*pc + par) + 4*t + g
    parts = []
    for m in range(NCORES):
        o = res.results[m]["out"].reshape(NB, NPC, G, OUT, 2, TB, W_DIM)
        parts.append(
            o.transpose(0, 3, 1, 4, 5, 2, 6).reshape(NB, OUT, H, W_DIM)
        )
    out = np.ascontiguousarray(np.concatenate(parts, axis=0)).astype(np.float32)
    if trace:
        kernel.last_results = res
    return out
